# revision 9
# baseline (speedup 1.0000x reference)
"""Trainium2 Bass kernel for the CN coupling-block problem (nn_CN_69312182223156).

Math (per subnet s on half-features x_s with conditioner c):
    h   = relu(c @ W1 + b1)                       # [B, 50]
    p   = h @ W2 + b2                             # [B, 9696]
    m1, b1p, m2 = p[:, :3200], p[:, 3200:6400], p[:, 6400:9600]   (viewed [B,32,100])
    bias2, eps, alpha = p[:, 9600:9632], p[:, 9632:9664]/10, p[:, 9664:]/10
    z   = x*m1 + b1p
    num = sum_l elu(z)*m2 ;  den = sum_l relu(-m1*m2) + 1
    y   = exp(alpha) * (x + 0.8*sigmoid(eps)*num/den) + bias2

Subnet 1: x=x1, c=x2.  Subnet 2: x=x2, c=y1.  Output concat([y1, y2]).

Strategy: pure data-parallel over 8 cores (2048 rows each), weights replicated.
Layout: batch on SBUF partitions (tiles of 128 rows). All matmuls on PE with
biases folded in via an appended ones-row in the stationary operand.  The big
[B, 9696] intermediate is produced in PSUM in 400-column chunks and consumed
immediately by a fused fp16 elementwise chain:
    elu(z)+1 = exp(min(z,0)) + relu(z)
    num = sum_l ((exp(min(z,0)) + relu(z)) - 1) * m2       (one STT op for the mul)
    den = sum_l relu(-m1*m2) + 1
ACT does the PSUM->SBUF f16 casts + exp; DVE does the fused scalar_tensor_tensor
/ tensor_scalar ops and the grouped free-axis reduces.
"""

import numpy as np

B = 16384
DIM = 32
LS = 100
NCORES = 8
BC = B // NCORES          # rows per core
NT = BC // 128            # 128-row tiles per core
DL = DIM * LS             # 3200
PW = 3 * DL + 3 * DIM     # 9696 params per row
CHUNK = 400               # params per PSUM chunk (4 dims x 100)
NCHUNK = DL // CHUNK      # 8
DPC = CHUNK // LS         # 4 dims per chunk

_cache = {}


def _build_program():
    import concourse.bass as bass
    import concourse.tile as tile
    import concourse.mybir as mybir
    from concourse import bacc, masks

    f32 = mybir.dt.float32
    f16 = mybir.dt.float16
    Alu = mybir.AluOpType
    Act = mybir.ActivationFunctionType
    X = mybir.AxisListType.X

    nc = bacc.Bacc("TRN2", target_bir_lowering=False)

    x_d = nc.dram_tensor("x", [BC, 2 * DIM], f32, kind="ExternalInput")
    w1a = [nc.dram_tensor(f"w1a{s}", [DIM + 1, 51], f16, kind="ExternalInput")
           for s in (1, 2)]
    w2a = [nc.dram_tensor(f"w2a{s}", [51, PW], f16, kind="ExternalInput")
           for s in (1, 2)]
    y_d = nc.dram_tensor("y", [BC, 2 * DIM], f32, kind="ExternalOutput")

    with tile.TileContext(nc) as tc:
        with (
            tc.tile_pool(name="const", bufs=1) as const,
            tc.tile_pool(name="io", bufs=3) as io,
            tc.tile_pool(name="mid", bufs=3) as mid,
            tc.tile_pool(name="ew", bufs=3) as ew,
            tc.tile_pool(name="tail", bufs=2) as tailp,
            tc.tile_pool(name="pmm", bufs=6, space="PSUM") as pmm,
            tc.tile_pool(name="psm", bufs=2, space="PSUM") as psm,
        ):
            # ---- constants ----
            w1s = []
            w2s = []
            for s in range(2):
                t1 = const.tile([DIM + 1, 51], f16, tag=f"w1_{s}")
                nc.sync.dma_start(t1, w1a[s][:])
                w1s.append(t1)
                t2 = const.tile([51, PW], f16, tag=f"w2_{s}")
                nc.sync.dma_start(t2, w2a[s][:])
                w2s.append(t2)
            ident = const.tile([128, 128], f16, tag="ident")
            masks.make_identity(nc, ident[:])

            for it in range(NT):
                r0 = it * 128
                xf = io.tile([128, 2 * DIM], f32, tag="xf")
                nc.sync.dma_start(xf, x_d[r0:r0 + 128, :])
                # f16 copy of x with a trailing ones column (for transposes)
                xh = io.tile([128, 2 * DIM + 1], f16, tag="xh")
                nc.vector.tensor_copy(xh[:, 0:2 * DIM], xf)
                nc.vector.memset(xh[:, 2 * DIM:], 1.0)

                # conditioner for subnet 1: [x2 | 1]^T  -> [33, 128]
                ct_ps = psm.tile([DIM + 1, 128], f16, tag="tp")
                nc.tensor.transpose(ct_ps, xh[:, DIM:2 * DIM + 1], ident)
                condT = mid.tile([DIM + 1, 128], f16, tag="condT")
                nc.scalar.copy(condT, ct_ps)

                y_out = io.tile([128, 2 * DIM], f32, tag="y_out")

                for s in range(2):
                    # h^T = relu(W1^T c^T + b1): [51, 128]; col 50 of W1aug is
                    # e_32 so row 50 comes out as relu(1) = 1 (the aug ones row).
                    h_ps = pmm.tile([51, 128], f32, tag="mm")
                    nc.tensor.matmul(h_ps, w1s[s], condT, start=True, stop=True)
                    hT = mid.tile([51, 128], f16, tag="hT")
                    nc.scalar.activation(hT, h_ps, Act.Relu)

                    xc32 = xf[:, s * DIM:(s + 1) * DIM]   # f32 x for this subnet
                    numden = ew.tile([128, 2, DIM], f32, tag="numden")

                    for c in range(NCHUNK):
                        co = c * CHUNK
                        m1p = pmm.tile([128, CHUNK], f32, tag="mm")
                        nc.tensor.matmul(m1p, hT, w2s[s][:, co:co + CHUNK],
                                         start=True, stop=True)
                        b1p = pmm.tile([128, CHUNK], f32, tag="mm")
                        nc.tensor.matmul(b1p, hT, w2s[s][:, DL + co:DL + co + CHUNK],
                                         start=True, stop=True)
                        m2p = pmm.tile([128, CHUNK], f32, tag="mm")
                        nc.tensor.matmul(m2p, hT, w2s[s][:, 2 * DL + co:2 * DL + co + CHUNK],
                                         start=True, stop=True)

                        m1s = ew.tile([128, CHUNK], f16, tag="m1s")
                        nc.scalar.copy(m1s, m1p)
                        b1s = ew.tile([128, CHUNK], f16, tag="b1s")
                        nc.scalar.copy(b1s, b1p)
                        m2s = ew.tile([128, CHUNK], f16, tag="m2s")
                        nc.scalar.copy(m2s, m2p)

                        # z = x*m1 + b1  (per-dim tensor_scalar for the x broadcast)
                        zmul = ew.tile([128, CHUNK], f16, tag="zmul")
                        zm3 = zmul.rearrange("p (d l) -> p d l", l=LS)
                        m1s3 = m1s.rearrange("p (d l) -> p d l", l=LS)
                        for j in range(DPC):
                            nc.vector.tensor_scalar_mul(
                                zm3[:, j, :], m1s3[:, j, :],
                                xc32[:, c * DPC + j:c * DPC + j + 1])
                        z = ew.tile([128, CHUNK], f16, tag="z")
                        nc.vector.tensor_add(z, zmul, b1s)
                        zn = ew.tile([128, CHUNK], f16, tag="zn")
                        nc.vector.tensor_scalar_min(zn, z, 0.0)
                        e = ew.tile([128, CHUNK], f16, tag="e")
                        nc.scalar.activation(e, zn, Act.Exp)
                        # w = relu(z) + exp(min(z,0)) = elu(z) + 1
                        w = ew.tile([128, CHUNK], f16, tag="w")
                        nc.vector.scalar_tensor_tensor(
                            w, in0=z, scalar=0.0, in1=e, op0=Alu.max, op1=Alu.add)
                        tr = ew.tile([128, 2, CHUNK], f16, tag="tr")
                        # t = (w - 1) * m2 = elu(z)*m2
                        nc.vector.scalar_tensor_tensor(
                            tr[:, 0, :], in0=w, scalar=-1.0, in1=m2s,
                            op0=Alu.add, op1=Alu.mult)
                        # u = (-m1) * m2 ; r = relu(u)
                        u = ew.tile([128, CHUNK], f16, tag="u")
                        nc.vector.scalar_tensor_tensor(
                            u, in0=m1s, scalar=-1.0, in1=m2s,
                            op0=Alu.mult, op1=Alu.mult)
                        nc.vector.tensor_scalar_max(tr[:, 1, :], u, 0.0)
                        # grouped reduce over l: [128, 2, 4, 100] -> [128, 2, 4]
                        tr4 = tr.rearrange("p t (d l) -> p t d l", l=LS)
                        nc.vector.tensor_reduce(
                            numden[:, :, c * DPC:(c + 1) * DPC], tr4, X, Alu.add)

                    # ---- tail ----
                    tp = pmm.tile([128, 3 * DIM], f32, tag="mm")
                    nc.tensor.matmul(tp, hT, w2s[s][:, 3 * DL:3 * DL + 3 * DIM],
                                     start=True, stop=True)
                    tps = tailp.tile([128, 3 * DIM], f32, tag="tps")
                    nc.scalar.copy(tps, tp)
                    b2p = tps[:, 0:DIM]
                    epp = tps[:, DIM:2 * DIM]
                    alp = tps[:, 2 * DIM:3 * DIM]

                    den = tailp.tile([128, DIM], f32, tag="den")
                    nc.vector.tensor_scalar_add(den, numden[:, 1, :], 1.0)
                    rec = tailp.tile([128, DIM], f32, tag="rec")
                    nc.vector.reciprocal_approx_fast(rec, den)
                    # sigmoid(eps/10) = 1 / (1 + exp(-eps/10))
                    nege = tailp.tile([128, DIM], f32, tag="nege")
                    nc.scalar.activation(nege, epp, Act.Exp, scale=-0.1)
                    sd = tailp.tile([128, DIM], f32, tag="sd")
                    nc.vector.tensor_scalar_add(sd, nege, 1.0)
                    sig = tailp.tile([128, DIM], f32, tag="sig")
                    nc.vector.reciprocal_approx_fast(sig, sd)
                    ea = tailp.tile([128, DIM], f32, tag="ea")
                    nc.scalar.activation(ea, alp, Act.Exp, scale=0.1)
                    frac = tailp.tile([128, DIM], f32, tag="frac")
                    nc.vector.tensor_mul(frac, numden[:, 0, :], rec)
                    q = tailp.tile([128, DIM], f32, tag="q")
                    nc.vector.scalar_tensor_tensor(
                        q, in0=frac, scalar=0.8, in1=sig, op0=Alu.mult, op1=Alu.mult)
                    sx = tailp.tile([128, DIM], f32, tag="sx")
                    nc.vector.tensor_add(sx, q, xc32)
                    yp = tailp.tile([128, DIM], f32, tag="yp")
                    nc.vector.tensor_mul(yp, ea, sx)
                    nc.vector.tensor_add(y_out[:, s * DIM:(s + 1) * DIM], yp, b2p)

                    if s == 0:
                        # conditioner for subnet 2: [y1 | 1]^T
                        y1h = mid.tile([128, DIM + 1], f16, tag="y1h")
                        nc.vector.tensor_copy(y1h[:, 0:DIM], y_out[:, 0:DIM])
                        nc.vector.memset(y1h[:, DIM:], 1.0)
                        c2_ps = psm.tile([DIM + 1, 128], f16, tag="tp")
                        nc.tensor.transpose(c2_ps, y1h, ident)
                        condT2 = mid.tile([DIM + 1, 128], f16, tag="condT2")
                        nc.scalar.copy(condT2, c2_ps)
                        condT = condT2

                nc.sync.dma_start(y_d[r0:r0 + 128, :], y_out)

    nc.compile()
    return nc


def _prep_weights(W1, b1, W2, b2):
    w1a = np.concatenate([W1, b1[None, :]], axis=0).astype(np.float16)  # [33, 50]
    ones_col = np.zeros((DIM + 1, 1), dtype=np.float16)
    ones_col[DIM, 0] = 1.0
    w1a = np.concatenate([w1a, ones_col], axis=1)                       # [33, 51]
    w2a = np.concatenate([W2, b2[None, :]], axis=0).astype(np.float16)  # [51, 9696]
    return np.ascontiguousarray(w1a), np.ascontiguousarray(w2a)


def kernel(**inputs):
    from concourse.bass_utils import run_bass_kernel_spmd

    if "nc" not in _cache:
        _cache["nc"] = _build_program()
    nc = _cache["nc"]

    x = np.ascontiguousarray(inputs["x"], dtype=np.float32)
    w1a1, w2a1 = _prep_weights(inputs["s1_W1"], inputs["s1_b1"],
                               inputs["s1_W2"], inputs["s1_b2"])
    w1a2, w2a2 = _prep_weights(inputs["s2_W1"], inputs["s2_b1"],
                               inputs["s2_W2"], inputs["s2_b2"])

    in_maps = []
    for i in range(NCORES):
        in_maps.append({
            "x": x[i * BC:(i + 1) * BC],
            "w1a1": w1a1, "w2a1": w2a1,
            "w1a2": w1a2, "w2a2": w2a2,
        })

    res = run_bass_kernel_spmd(nc, in_maps, core_ids=list(range(NCORES)),
                               **_cache.get("run_kwargs", {}))
    out = np.concatenate([r["y"] for r in res.results], axis=0)
    _cache["last_results"] = res
    return out


# revision 24
# speedup vs baseline: 1.5051x; 1.5051x over previous
"""Trainium2 Bass kernel for the CN coupling-block problem (nn_CN_69312182223156).

Math (per subnet s on half-features x_s with conditioner c):
    h   = relu(c @ W1 + b1)                       # [B, 50]
    p   = h @ W2 + b2                             # [B, 9696]
    m1, b1p, m2 = p[:, :3200], p[:, 3200:6400], p[:, 6400:9600]   (viewed [B,32,100])
    bias2, eps, alpha = p[:, 9600:9632], p[:, 9632:9664]/10, p[:, 9664:]/10
    z   = x*m1 + b1p
    num = sum_l elu(z)*m2 ;  den = sum_l relu(-m1*m2) + 1
    y   = exp(alpha) * (x + 0.8*sigmoid(eps)*num/den) + bias2

Subnet 1: x=x1, c=x2.  Subnet 2: x=x2, c=y1.  Output concat([y1, y2]).

Strategy: pure data-parallel over 8 cores (2048 rows each), weights replicated.
Layout: batch on SBUF partitions (tiles of 128 rows). All matmuls on PE with
biases folded in via augmented weights (extra ones-row/column), including
S2 = sum_l mat2 as 32 extra output columns so `num` needs no -1 term:
    elu(z)+1 = exp(min(z,0)) + relu(z)
    num = sum_l (elu(z)+1)*m2 - S2 ;  den = sum_l relu(-m1*m2) + 1
The [B, 9696] intermediate is produced into paired PSUM banks in 800-column
chunks and consumed immediately: ScalarE does one PSUM->SBUF f16 cast per
stream chunk plus exp and relu(-m1*m2); VectorE does the per-dim x-broadcast
tensor_scalar (4x rate), the f16 tensor_tensor products (2x rate), and a
folded reduction (two 2x-rate pair-adds, then a 1x tensor_reduce over 25).
Issue order is phase-split (all subnet-1 tiles, then all subnet-2 tiles) so
the scheduler always has independent work around the y1 dependency.
Cost-model exec time: ~631 us/core; both vector engines ~93% busy.
"""

import numpy as np

B = 16384
DIM = 32
LS = 100
NCORES = 8
BC = B // NCORES          # rows per core
NT = BC // 128            # 128-row tiles per core
DL = DIM * LS             # 3200
PW = 3 * DL + 3 * DIM     # 9696 params per row
CHUNK = 800               # params per elementwise chunk (8 dims x 100)
HALF = 400                # params per PSUM-bank matmul
NCHUNK = DL // CHUNK      # 4
DPC = CHUNK // LS         # 8 dims per chunk

_cache = {}


def _build_program():
    import concourse.bass as bass
    import concourse.tile as tile
    import concourse.mybir as mybir
    from concourse import bacc, masks

    f32 = mybir.dt.float32
    f16 = mybir.dt.float16
    Alu = mybir.AluOpType
    Act = mybir.ActivationFunctionType
    X = mybir.AxisListType.X

    nc = bacc.Bacc("TRN2", target_bir_lowering=False)

    x_d = nc.dram_tensor("x", [BC, 2 * DIM], f32, kind="ExternalInput")
    w1a = [nc.dram_tensor(f"w1a{s}", [DIM + 1, 51], f16, kind="ExternalInput")
           for s in (1, 2)]
    w2a = [nc.dram_tensor(f"w2a{s}", [51, PW + DIM], f16, kind="ExternalInput")
           for s in (1, 2)]
    y_d = nc.dram_tensor("y", [BC, 2 * DIM], f32, kind="ExternalOutput")

    with tile.TileContext(nc) as tc:
        with (
            tc.tile_pool(name="const", bufs=1) as const,
            tc.tile_pool(name="io", bufs=4) as io,
            tc.tile_pool(name="mid", bufs=4) as mid,
            tc.tile_pool(name="ew", bufs=6) as ew,
            tc.tile_pool(name="tail", bufs=3) as tailp,
            tc.tile_pool(name="pmm", bufs=8, space="PSUM") as pmm,
        ):
            # ---- constants ----
            w1s = []
            w2s = []
            for s in range(2):
                t1 = const.tile([DIM + 1, 51], f16, tag=f"w1_{s}")
                nc.sync.dma_start(t1, w1a[s][:])
                w1s.append(t1)
                t2 = const.tile([51, PW + DIM], f16, tag=f"w2_{s}")
                nc.sync.dma_start(t2, w2a[s][:])
                w2s.append(t2)
            ident = const.tile([128, 128], f16, tag="ident")
            masks.make_identity(nc, ident[:])
            identf = const.tile([128, 128], f32, tag="identf")
            masks.make_identity(nc, identf[:])

            for it in range(NT):
                r0 = it * 128
                xf = io.tile([128, 2 * DIM], f32, tag="xf")
                nc.sync.dma_start(xf, x_d[r0:r0 + 128, :])
                # f16 copy of x with a trailing ones column (for transposes)
                xh = io.tile([128, 2 * DIM + 1], f16, tag="xh")
                nc.vector.tensor_copy(xh[:, 0:2 * DIM], xf)
                nc.vector.memset(xh[:, 2 * DIM:], 1.0)

                # conditioner for subnet 1: [x2 | 1]^T  -> [33, 128]
                ct_ps = pmm.tile([DIM + 1, 128], f16, tag="mm")
                nc.tensor.transpose(ct_ps, xh[:, DIM:2 * DIM + 1], ident)
                condT = mid.tile([DIM + 1, 128], f16, tag="condT")
                nc.scalar.copy(condT, ct_ps)

                y_out = io.tile([128, 2 * DIM], f32, tag="y_out")

                for s in range(2):
                    # h^T = relu(W1^T c^T + b1): [51, 128]; col 50 of W1aug is
                    # e_32 so row 50 comes out as relu(1) = 1 (the aug ones row).
                    h_ps = psm.tile([51, 128], f32, tag="tp")
                    nc.tensor.matmul(h_ps, w1s[s], condT, start=True, stop=True)
                    hT = mid.tile([51, 128], f16, tag="hT")
                    nc.scalar.activation(hT, h_ps, Act.Relu)

                    xc32 = xf[:, s * DIM:(s + 1) * DIM]   # f32 x for this subnet
                    numden = ew.tile([128, 2, DIM], f32, tag="numden")

                    for c in range(NCHUNK):
                        co = c * CHUNK
                        m1s = ew.tile([128, CHUNK], f16, tag="m1s")
                        b1s = ew.tile([128, CHUNK], f16, tag="b1s")
                        m2s = ew.tile([128, CHUNK], f16, tag="m2s")
                        for (dst, base) in ((m1s, 0), (b1s, DL), (m2s, 2 * DL)):
                            for hh in range(CHUNK // HALF):
                                o = base + co + hh * HALF
                                mp = pmm.tile([128, HALF], f32, tag="mm")
                                nc.tensor.matmul(mp, hT, w2s[s][:, o:o + HALF],
                                                 start=True, stop=True)
                                nc.scalar.copy(dst[:, hh * HALF:(hh + 1) * HALF], mp)

                        # z = x*m1 + b1  (per-dim tensor_scalar for the x broadcast)
                        zmul = ew.tile([128, CHUNK], f16, tag="zmul")
                        zm3 = zmul.rearrange("p (d l) -> p d l", l=LS)
                        m1s3 = m1s.rearrange("p (d l) -> p d l", l=LS)
                        for j in range(DPC):
                            nc.vector.tensor_scalar_mul(
                                zm3[:, j, :], m1s3[:, j, :],
                                xc32[:, c * DPC + j:c * DPC + j + 1])
                        z = ew.tile([128, CHUNK], f16, tag="z")
                        nc.vector.tensor_add(z, zmul, b1s)
                        zn = ew.tile([128, CHUNK], f16, tag="zn")
                        nc.vector.tensor_scalar_min(zn, z, 0.0)
                        zp = ew.tile([128, CHUNK], f16, tag="zp")
                        nc.vector.tensor_scalar_max(zp, z, 0.0)
                        e = ew.tile([128, CHUNK], f16, tag="e")
                        nc.scalar.activation(e, zn, Act.Exp)
                        # w = relu(z) + exp(min(z,0)) = elu(z) + 1
                        w = ew.tile([128, CHUNK], f16, tag="w")
                        nc.vector.tensor_add(w, zp, e)
                        tr = ew.tile([128, 2, CHUNK], f16, tag="tr")
                        # t = w*m2 = (elu(z)+1)*m2; sum_l m2 (S2) subtracted in tail
                        nc.vector.tensor_mul(tr[:, 0, :], w, m2s)
                        # u = m1*m2 ; r = relu(-u) on ACT
                        u = ew.tile([128, CHUNK], f16, tag="u")
                        nc.vector.tensor_mul(u, m1s, m2s)
                        nc.scalar.activation(tr[:, 1, :], u, Act.Relu, scale=-1.0)
                        # grouped reduce over l: [128, 2, 4, 100] -> [128, 2, 4]
                        tr4 = tr.rearrange("p t (d l) -> p t d l", l=LS)
                        nc.vector.tensor_reduce(
                            numden[:, :, c * DPC:(c + 1) * DPC], tr4, X, Alu.add)

                    # ---- tail (bias2 | eps | alpha | S2) ----
                    tp = psm.tile([128, 4 * DIM], f32, tag="tp")
                    nc.tensor.matmul(tp, hT, w2s[s][:, 3 * DL:3 * DL + 4 * DIM],
                                     start=True, stop=True)
                    tps = tailp.tile([128, 4 * DIM], f32, tag="tps")
                    nc.scalar.copy(tps, tp)
                    b2p = tps[:, 0:DIM]
                    epp = tps[:, DIM:2 * DIM]
                    alp = tps[:, 2 * DIM:3 * DIM]
                    s2p = tps[:, 3 * DIM:4 * DIM]

                    den = tailp.tile([128, DIM], f32, tag="den")
                    nc.vector.tensor_scalar_add(den, numden[:, 1, :], 1.0)
                    rec = tailp.tile([128, DIM], f32, tag="rec")
                    nc.vector.reciprocal_approx_fast(rec, den)
                    # sigmoid(eps/10) = 1 / (1 + exp(-eps/10))
                    nege = tailp.tile([128, DIM], f32, tag="nege")
                    nc.scalar.activation(nege, epp, Act.Exp, scale=-0.1)
                    sd = tailp.tile([128, DIM], f32, tag="sd")
                    nc.vector.tensor_scalar_add(sd, nege, 1.0)
                    sig = tailp.tile([128, DIM], f32, tag="sig")
                    nc.vector.reciprocal_approx_fast(sig, sd)
                    ea = tailp.tile([128, DIM], f32, tag="ea")
                    nc.scalar.activation(ea, alp, Act.Exp, scale=0.1)
                    nums = tailp.tile([128, DIM], f32, tag="nums")
                    nc.vector.tensor_sub(nums, numden[:, 0, :], s2p)
                    frac = tailp.tile([128, DIM], f32, tag="frac")
                    nc.vector.tensor_mul(frac, nums, rec)
                    q = tailp.tile([128, DIM], f32, tag="q")
                    nc.vector.scalar_tensor_tensor(
                        q, in0=frac, scalar=0.8, in1=sig, op0=Alu.mult, op1=Alu.mult)
                    sx = tailp.tile([128, DIM], f32, tag="sx")
                    nc.vector.tensor_add(sx, q, xc32)
                    yp = tailp.tile([128, DIM], f32, tag="yp")
                    nc.vector.tensor_mul(yp, ea, sx)
                    nc.vector.tensor_add(y_out[:, s * DIM:(s + 1) * DIM], yp, b2p)

                    if s == 0:
                        # conditioner for subnet 2: [y1 | 1]^T
                        y1h = mid.tile([128, DIM + 1], f16, tag="y1h")
                        nc.vector.tensor_copy(y1h[:, 0:DIM], y_out[:, 0:DIM])
                        nc.vector.memset(y1h[:, DIM:], 1.0)
                        c2_ps = pmm.tile([DIM + 1, 128], f16, tag="mm")
                        nc.tensor.transpose(c2_ps, y1h, ident)
                        condT2 = mid.tile([DIM + 1, 128], f16, tag="condT2")
                        nc.scalar.copy(condT2, c2_ps)
                        condT = condT2

                nc.sync.dma_start(y_d[r0:r0 + 128, :], y_out)

    nc.compile()
    return nc


def _prep_weights(W1, b1, W2, b2):
    w1a = np.concatenate([W1, b1[None, :]], axis=0).astype(np.float16)  # [33, 50]
    ones_col = np.zeros((DIM + 1, 1), dtype=np.float16)
    ones_col[DIM, 0] = 1.0
    w1a = np.concatenate([w1a, ones_col], axis=1)                       # [33, 51]
    w2a = np.concatenate([W2, b2[None, :]], axis=0)                     # [51, 9696] f32
    # append S2 columns: S2[:, d] = sum_l w2a[:, mat2 region (d, l)]
    m2cols = w2a[:, 2 * DL:3 * DL].reshape(51, DIM, LS)
    s2 = m2cols.sum(axis=2)                                             # [51, DIM]
    w2a = np.concatenate([w2a, s2], axis=1).astype(np.float16)          # [51, 9728]
    return np.ascontiguousarray(w1a), np.ascontiguousarray(w2a)


def kernel(**inputs):
    from concourse.bass_utils import run_bass_kernel_spmd

    if "nc" not in _cache:
        _cache["nc"] = _build_program()
    nc = _cache["nc"]

    x = np.ascontiguousarray(inputs["x"], dtype=np.float32)
    w1a1, w2a1 = _prep_weights(inputs["s1_W1"], inputs["s1_b1"],
                               inputs["s1_W2"], inputs["s1_b2"])
    w1a2, w2a2 = _prep_weights(inputs["s2_W1"], inputs["s2_b1"],
                               inputs["s2_W2"], inputs["s2_b2"])

    in_maps = []
    for i in range(NCORES):
        in_maps.append({
            "x": x[i * BC:(i + 1) * BC],
            "w1a1": w1a1, "w2a1": w2a1,
            "w1a2": w1a2, "w2a2": w2a2,
        })

    last_err = None
    for attempt in range(3):
        try:
            res = run_bass_kernel_spmd(nc, in_maps, core_ids=list(range(NCORES)),
                                       **_cache.get("run_kwargs", {}))
            out = np.concatenate([r["y"] for r in res.results], axis=0)
            _cache["last_results"] = res
            return out
        except Exception as ex:  # transient NRT/device errors: retry
            last_err = ex
    raise last_err
